# revision 14
# baseline (speedup 1.0000x reference)
"""ConvexSoftMixer Trainium2 kernel (v3).

Shards batch*heads (1*8 = 8) across 8 NeuronCores, one head per core.

Math (exact refactor of the reference; the reference's per-row m1 cancels
analytically, and its per-column m2 is replaced by one scalar stabilizer
m = max_t gk[t], which shifts exp args by at most max|u| ~ 5.4):

    z1 = softplus(W1sp qk + b1)            (q and k stacked on 128 partitions,
    l2 = ln(1 + exp(-(W2sp z1 + b2)))       block-diagonal weights)
    z2 = (W2sp z1 + b2) + l2               (softplus, valid since z2_pre >= 0;
                                            the Exp/Ln LUTs clamp near x=41 and
                                            z2_pre reaches ~48, so the -x form
                                            keeps both LUT args small)
    fq[s] = sum_f z2q[f,s] ; gk[t] = sum_f z2k[f,t]
    m = max_t gk[t]
    pk'[t,r] = exp(k_t . Wh_r + gk[t] - m)
    eu[t,p]  = exp(v_t . Wv_p)
    M[r,p]   = sum_t pk'[t,r] eu[t,p]
    phiq[r,s] = exp(q_s . Wh_r)
    A^T[s,p] = sum_r phiq[r,s] M[r,p]
    y[s,p]   = ln(A^T[s,p]) + fq[s] + m - ln(S)

Layout tricks:
  * fq/gk are produced directly as per-t COLUMNS (fg[t, (fq,gk)] chunks):
    since fq = w2sum^T z1 + sum(b2) + 1^T l2 is linear, it is accumulated
    by three matmuls per s-chunk with lhsT = z1 / ones-row / l2 and
    rhs = (w2sum | b2sum | eqek) columns. gk lands in the per-partition
    form the pk' exp-bias needs, fq in the form the final y add needs —
    no transposes, no broadcast rank-1s in the tail.
  * All matmul operands are bf16 (1 PE cycle/row vs 4 for fp32);
    accumulation stays fp32 in PSUM.
  * A pre-placed InstLoadActFuncSet selects the combined Exp+Ln table:
    zero activation-table reloads (the fp32 baseline spent 7.7us on 6).
  * pk'/lnA/A^T chunks live in per-chunk tiles so tile-granular WAR
    deps don't serialize the pk'->M and at->ln->add tails.
"""

import math

import numpy as np

_B, _H, _S, _D, _P = 1, 8, 512, 64, 32
_NCORES = 8
_LN_S = math.log(float(_S))

_CACHE = {}

# wts column map (bf16, [128, 456]):
_W1 = 0         # [:, 0:128]   w1 block-diag (lhsT: [in_feat, out_feat])
_W2 = 128       # [:, 128:256] w2 block-diag
_EQEK = 256     # [:, 256:258] eq | ek indicator columns
_WH = 258       # [0:64, 258:290]  Wh.T
_WV = 290       # [0:64, 290:322]  Wv.T
_B1 = 322       # [:, 322] b1 ; [:, 323] -b2 (bias of the -x exp)
_W2S = 324      # [:, 324:326] w2 column sums (q rows 0:64 col 0, k col 1)
_ONES = 326     # [0, 326:454] ones row ; [0, 454:456] (sum b2q, sum b2k)
_WTSW = 456


def _build_bass(dump=False):
    import concourse.tile as tile
    from concourse import bacc, mybir
    from concourse.bass_isa import ReduceOp
    from concourse.alu_op_type import AluOpType as OP

    f32 = mybir.dt.float32
    bf16 = mybir.dt.bfloat16
    AF = mybir.ActivationFunctionType
    AX = mybir.AxisListType.X

    nc = bacc.Bacc("TRN2", target_bir_lowering=False, debug=False)

    wts_d = nc.dram_tensor("wts", [128, _WTSW], bf16, kind="ExternalInput").ap()
    xqk_d = nc.dram_tensor("xqk", [128, _S], bf16, kind="ExternalInput").ap()
    vkt_d = nc.dram_tensor("vkt", [_D, 2 * _S], bf16, kind="ExternalInput").ap()
    y_d = nc.dram_tensor("y", [128, 4 * _P], f32, kind="ExternalOutput").ap()

    NCH = _S // 128  # 4 sequence chunks of 128

    with tile.TileContext(nc) as tc:
        with (
            tc.tile_pool(name="pin", bufs=1) as pin,
            tc.tile_pool(name="pwork", bufs=1) as pw,
            tc.tile_pool(name="psA", bufs=1, space="PSUM") as psA,  # z1,z2
            tc.tile_pool(name="psB", bufs=2, space="PSUM") as psB,  # phiq,fg,M
            tc.tile_pool(name="psC", bufs=1, space="PSUM") as psC,  # pk|u
            tc.tile_pool(name="psD", bufs=4, space="PSUM") as psD,  # at chunks
        ):
            # ---- input DMAs: xqk on the scalar queue (free earliest),
            # wts on sync, vkt on gpsimd; act-table preload after the issue.
            nc.scalar.add_instruction(
                mybir.InstLoadActFuncSet(
                    name="preload_act_nle", ins=[], outs=[], act_func_set_id=6
                )
            )
            xqk = pin.tile([128, _S], bf16, tag="xqk")
            nc.scalar.dma_start(out=xqk, in_=xqk_d)
            wts = pin.tile([128, _WTSW], bf16, tag="wts")
            nc.sync.dma_start(out=wts, in_=wts_d)
            vkt = pin.tile([_D, 2 * _S], bf16, tag="vkt")
            nc.gpsimd.dma_start(out=vkt, in_=vkt_d)

            w1 = wts[:, _W1:_W1 + 128]
            w2 = wts[:, _W2:_W2 + 128]
            eqek = wts[:, _EQEK:_EQEK + 2]
            wh_t = wts[0:_D, _WH:_WH + _P]
            wv_t = wts[0:_D, _WV:_WV + _P]
            b1col = wts[:, _B1:_B1 + 1]
            nb2col = wts[:, _B1 + 1:_B1 + 2]
            w2s = wts[:, _W2S:_W2S + 2]
            ones_row = wts[0:1, _ONES:_ONES + 128]
            b2s_row = wts[0:1, _ONES + 128:_ONES + 130]

            # ---- PE: z1, phiq, u, pk (only need inputs) ----
            z1_p = psA.tile([128, _S], f32, tag="big")
            nc.tensor.matmul(out=z1_p, lhsT=w1, rhs=xqk, start=True, stop=True)

            phiq_p = psB.tile([_P, _S], f32, tag="mid")
            nc.tensor.matmul(out=phiq_p, lhsT=wh_t, rhs=xqk[0:_D, :],
                             start=True, stop=True)

            # ---- ICNN layer 1 on ACT: z1 = ln(1 + exp(x + b1)) ----
            e1 = pw.tile([128, _S], bf16, tag="e1")
            nc.scalar.activation(out=e1, in_=z1_p, func=AF.Exp, bias=b1col, scale=1.0)
            z1 = pw.tile([128, _S], bf16, tag="z1")
            nc.scalar.activation(out=z1, in_=e1, func=AF.Ln, bias=1.0, scale=1.0)

            # ---- layer 2: only l2 = ln(1 + e^-(x+b2)) is nonlinear; the
            # linear part of z2 flows into fq/gk via the w2sum matmuls.
            z2_p = psA.tile([128, _S], f32, tag="big")
            nc.tensor.matmul(out=z2_p, lhsT=w2, rhs=z1, start=True, stop=True)

            # pku: cols 0:128 = u chunks, 128:256 = pk chunks
            pku_p = psC.tile([128, 2 * 128], f32, tag="pku")
            for c in range(NCH):
                nc.tensor.matmul(
                    out=pku_p[:, c * _P:(c + 1) * _P],
                    lhsT=vkt[:, c * 128:(c + 1) * 128],
                    rhs=wv_t, start=True, stop=True,
                )
            for c in range(NCH):
                nc.tensor.matmul(
                    out=pku_p[:, 128 + c * _P:128 + (c + 1) * _P],
                    lhsT=vkt[:, _S + c * 128:_S + (c + 1) * 128],
                    rhs=wh_t, start=True, stop=True,
                )


            # phiq exp rides the ACT idle slot while the z2 matmul runs
            phiq = pw.tile([_P, _S], bf16, tag="phiq")
            nc.scalar.activation(out=phiq, in_=phiq_p, func=AF.Exp,
                                 bias=0.0, scale=1.0)

            e2n = pw.tile([128, _S], bf16, tag="e2n")
            nc.scalar.activation(out=e2n, in_=z2_p, func=AF.Exp, bias=nb2col,
                                 scale=-1.0)
            l2 = pw.tile([128, _S], bf16, tag="l2")
            nc.scalar.activation(out=l2, in_=e2n, func=AF.Ln, bias=1.0, scale=1.0)

            eu = pw.tile([128, 128], bf16, tag="eu")
            nc.scalar.activation(out=eu, in_=pku_p[:, 0:128], func=AF.Exp,
                                 bias=0.0, scale=1.0)

            # ---- fq/gk columns: fg[t, 2c+(0=fq,1=gk)] ----
            # per chunk: w2sum^T z1 + ones (x) b2sums + eqek^T l2
            # all z1-part matmuls are emitted before the l2-part ones so the
            # in-order PE queue doesn't stall on l2 mid-loop; the per-chunk
            # accumulation groups therefore interleave (safe on HW: PSUM
            # accumulate is per-instruction), hence skip_group_check.
            fg_p = psB.tile([128, 2 * NCH], f32, tag="mid")
            for c in range(NCH):
                nc.tensor.matmul(
                    out=fg_p[:, 2 * c:2 * c + 2],
                    lhsT=z1[:, c * 128:(c + 1) * 128],
                    rhs=w2s, start=True, stop=False,
                )
                nc.tensor.matmul(
                    out=fg_p[:, 2 * c:2 * c + 2],
                    lhsT=l2[:, c * 128:(c + 1) * 128],
                    rhs=eqek, start=False, stop=True,
                )

            # ---- m = max_t gk; gkm = gk - m; fqm = fq + m - ln(S) ----
            m1 = pw.tile([128, 1], f32, tag="m1")
            nc.vector.reduce_max(m1, fg_p[:, 1:2 * NCH:2], axis=AX)
            allmax = pw.tile([128, 1], f32, tag="allmax")
            nc.gpsimd.partition_all_reduce(allmax, m1, channels=128,
                                           reduce_op=ReduceOp.max)
            gkm = pw.tile([128, NCH], f32, tag="gkm")
            nc.vector.tensor_scalar(gkm, fg_p[:, 1:2 * NCH:2], allmax, None,
                                    op0=OP.subtract)
            fqm = pw.tile([128, NCH], f32, tag="fqm")
            nc.vector.tensor_scalar(fqm, fg_p[:, 0:2 * NCH:2], allmax, -_LN_S,
                                    op0=OP.add, op1=OP.add)

            # ---- pk' = exp(pk_pre + (gk - m)) ; M = sum_t pk' eu ----
            # per-chunk pk' tiles so ACT writes don't WAR-serialize with
            # PE reads of earlier chunks
            M_p = psB.tile([_P, _P], f32, tag="mid")
            pk2 = [pw.tile([128, _P], bf16, tag=f"pk2_{c}", name=f"pk2_{c}")
                   for c in range(NCH)]
            for c in range(NCH):
                nc.scalar.activation(
                    out=pk2[c],
                    in_=pku_p[:, 128 + c * _P:128 + (c + 1) * _P],
                    func=AF.Exp, bias=gkm[:, c:c + 1], scale=1.0,
                )
                nc.tensor.matmul(
                    out=M_p, lhsT=pk2[c], rhs=eu[:, c * _P:(c + 1) * _P],
                    start=(c == 0), stop=(c == NCH - 1),
                )
            M_sb = pw.tile([_P, _P], bf16, tag="M_sb")
            nc.vector.tensor_copy(out=M_sb, in_=M_p)

            # ---- A^T chunks, ln, + (fq + m - ln S), per-chunk tiles ----
            y_sb0 = pw.tile([128, 2 * _P], f32, tag="y0")
            y_sb1 = pw.tile([128, 2 * _P], f32, tag="y1")
            y_halves = [y_sb0, y_sb0, y_sb1, y_sb1]
            for c in range(NCH):
                at_c = psD.tile([128, _P], f32, tag="at", name=f"at_{c}")
                nc.tensor.matmul(
                    out=at_c,
                    lhsT=phiq[:, c * 128:(c + 1) * 128],
                    rhs=M_sb, start=True, stop=True,
                )
                lnA_c = pw.tile([128, _P], f32, tag=f"lnA_{c}", name=f"lnA_{c}")
                nc.scalar.activation(out=lnA_c, in_=at_c, func=AF.Ln,
                                     bias=0.0, scale=1.0)
                nc.vector.tensor_scalar(
                    y_halves[c][:, (c % 2) * _P:(c % 2 + 1) * _P], lnA_c,
                    fqm[:, c:c + 1], None, op0=OP.add,
                )

            nc.sync.dma_start(out=y_d[:, 0:2 * _P], in_=y_sb0)
            nc.gpsimd.dma_start(out=y_d[:, 2 * _P:4 * _P], in_=y_sb1)

            if dump:
                for nm, t in [
                    ("d_z1", z1), ("d_l2", l2), ("d_eu", eu),
                    ("d_phiq", phiq), ("d_gkm", gkm), ("d_fqm", fqm),
                    ("d_Msb", M_sb), ("d_m1", m1), ("d_allmax", allmax),
                ]:
                    dd = nc.dram_tensor(nm, list(t.shape), t.dtype,
                                        kind="ExternalOutput").ap()
                    nc.sync.dma_start(out=dd, in_=t)

    if not nc.is_finalized():
        nc.finalize()
    return nc


def _host_inputs(q, k, v, spW1q, b1q, spW2q, b2q, spW1k, b1k, spW2k, b2k, Wh, Wv):
    """Build the per-core input maps (numpy layout prep only)."""
    import ml_dtypes

    bf16 = ml_dtypes.bfloat16
    S, D, P = _S, _D, _P

    wts = np.zeros((128, _WTSW), np.float32)
    wts[0:D, _W1:_W1 + D] = spW1q.T
    wts[D:2 * D, _W1 + D:_W1 + 2 * D] = spW1k.T
    wts[0:D, _W2:_W2 + D] = spW2q.T
    wts[D:2 * D, _W2 + D:_W2 + 2 * D] = spW2k.T
    wts[0:D, _EQEK] = 1.0
    wts[D:2 * D, _EQEK + 1] = 1.0
    wts[0:D, _WH:_WH + P] = Wh.T
    wts[0:D, _WV:_WV + P] = Wv.T
    wts[0:D, _B1] = b1q
    wts[D:2 * D, _B1] = b1k
    wts[0:D, _B1 + 1] = -b2q
    wts[D:2 * D, _B1 + 1] = -b2k
    wts[0:D, _W2S] = spW2q.sum(axis=0)      # column sums of W2q
    wts[D:2 * D, _W2S + 1] = spW2k.sum(axis=0)
    wts[0, _ONES:_ONES + 128] = 1.0
    wts[0, _ONES + 128] = float(np.sum(b2q))
    wts[0, _ONES + 129] = float(np.sum(b2k))
    wts = wts.astype(bf16)

    in_maps = []
    for h in range(_H):
        xqk = np.empty((128, S), np.float32)
        xqk[0:D] = q[0, h].T
        xqk[D:2 * D] = k[0, h].T
        vkt = np.empty((D, 2 * S), np.float32)
        vkt[:, 0:S] = v[0, h].T
        vkt[:, S:2 * S] = k[0, h].T  # kT copy at partitions 0:64 for pk lhsT
        in_maps.append(dict(
            wts=wts,
            xqk=xqk.astype(bf16),
            vkt=vkt.astype(bf16),
        ))
    return in_maps


def kernel(**inputs):
    from concourse.bass_utils import run_bass_kernel_spmd

    np_in = {k: np.asarray(v) for k, v in inputs.items()}
    q, k, v = np_in["q"], np_in["k"], np_in["v"]

    def sp(x):  # softplus for the small weight matrices (host prep)
        return np.log1p(np.exp(x.astype(np.float64))).astype(np.float32)

    in_maps = _host_inputs(
        q, k, v,
        sp(np_in["sq_raw1"]), np_in["sq_b1"], sp(np_in["sq_raw2"]), np_in["sq_b2"],
        sp(np_in["sk_raw1"]), np_in["sk_b1"], sp(np_in["sk_raw2"]), np_in["sk_b2"],
        np_in["Wh"], np_in["Wv"],
    )

    if "nc" not in _CACHE:
        _CACHE["nc"] = _build_bass()
    nc = _CACHE["nc"]

    res = run_bass_kernel_spmd(nc, in_maps, list(range(_NCORES)))
    out = np.zeros((_B, _H, _S, _P), np.float32)
    for h in range(_H):
        yt = res.results[h]["y"]  # [128, 4*P]; col block c = y[c*128:(c+1)*128, :]
        out[0, h] = yt.reshape(128, 4, _P).transpose(1, 0, 2).reshape(_S, _P)
    return out


# revision 15
# speedup vs baseline: 1.0243x; 1.0243x over previous
"""ConvexSoftMixer Trainium2 kernel (v3).

Shards batch*heads (1*8 = 8) across 8 NeuronCores, one head per core.

Math (exact refactor of the reference; the reference's per-row m1 cancels
analytically, and its per-column m2 is replaced by one scalar stabilizer
m = max_t gk[t], which shifts exp args by at most max|u| ~ 5.4):

    z1 = softplus(W1sp qk + b1)            (q and k stacked on 128 partitions,
    l2 = ln(1 + exp(-(W2sp z1 + b2)))       block-diagonal weights)
    z2 = (W2sp z1 + b2) + l2               (softplus, valid since z2_pre >= 0;
                                            the Exp/Ln LUTs clamp near x=41 and
                                            z2_pre reaches ~48, so the -x form
                                            keeps both LUT args small)
    fq[s] = sum_f z2q[f,s] ; gk[t] = sum_f z2k[f,t]
    m = max_t gk[t]
    pk'[t,r] = exp(k_t . Wh_r + gk[t] - m)
    eu[t,p]  = exp(v_t . Wv_p)
    M[r,p]   = sum_t pk'[t,r] eu[t,p]
    phiq[r,s] = exp(q_s . Wh_r)
    A^T[s,p] = sum_r phiq[r,s] M[r,p]
    y[s,p]   = ln(A^T[s,p]) + fq[s] + m - ln(S)

Layout tricks:
  * fq/gk are produced directly as per-t COLUMNS (fg[t, (fq,gk)] chunks):
    since fq = w2sum^T z1 + sum(b2) + 1^T l2 is linear, it is accumulated
    by three matmuls per s-chunk with lhsT = z1 / ones-row / l2 and
    rhs = (w2sum | b2sum | eqek) columns. gk lands in the per-partition
    form the pk' exp-bias needs, fq in the form the final y add needs —
    no transposes, no broadcast rank-1s in the tail.
  * All matmul operands are bf16 (1 PE cycle/row vs 4 for fp32);
    accumulation stays fp32 in PSUM.
  * A pre-placed InstLoadActFuncSet selects the combined Exp+Ln table:
    zero activation-table reloads (the fp32 baseline spent 7.7us on 6).
  * pk'/lnA/A^T chunks live in per-chunk tiles so tile-granular WAR
    deps don't serialize the pk'->M and at->ln->add tails.
"""

import math

import numpy as np

_B, _H, _S, _D, _P = 1, 8, 512, 64, 32
_NCORES = 8
_LN_S = math.log(float(_S))

_CACHE = {}

# wts column map (bf16, [128, 456]):
_W1 = 0         # [:, 0:128]   w1 block-diag (lhsT: [in_feat, out_feat])
_W2 = 128       # [:, 128:256] w2 block-diag
_EQEK = 256     # [:, 256:258] eq | ek indicator columns
_WH = 258       # [0:64, 258:290]  Wh.T
_WV = 290       # [0:64, 290:322]  Wv.T
_B1 = 322       # [:, 322] b1 ; [:, 323] -b2 (bias of the -x exp)
_W2S = 324      # [:, 324:326] w2 column sums (q rows 0:64 col 0, k col 1)
_ONES = 326     # [0, 326:454] ones row ; [0, 454:456] (sum b2q, sum b2k)
_WTSW = 456


def _build_bass(dump=False):
    import concourse.tile as tile
    from concourse import bacc, mybir
    from concourse.bass_isa import ReduceOp
    from concourse.alu_op_type import AluOpType as OP

    f32 = mybir.dt.float32
    bf16 = mybir.dt.bfloat16
    AF = mybir.ActivationFunctionType
    AX = mybir.AxisListType.X

    nc = bacc.Bacc("TRN2", target_bir_lowering=False, debug=False)

    wts_d = nc.dram_tensor("wts", [128, _WTSW], bf16, kind="ExternalInput").ap()
    xqk_d = nc.dram_tensor("xqk", [128, _S], bf16, kind="ExternalInput").ap()
    vkt_d = nc.dram_tensor("vkt", [_D, 2 * _S], bf16, kind="ExternalInput").ap()
    y_d = nc.dram_tensor("y", [128, 4 * _P], f32, kind="ExternalOutput").ap()

    NCH = _S // 128  # 4 sequence chunks of 128

    with tile.TileContext(nc) as tc:
        with (
            tc.tile_pool(name="pin", bufs=1) as pin,
            tc.tile_pool(name="pwork", bufs=1) as pw,
            tc.tile_pool(name="psA", bufs=1, space="PSUM") as psA,  # z1,z2
            tc.tile_pool(name="psB", bufs=2, space="PSUM") as psB,  # phiq,fg,M
            tc.tile_pool(name="psC", bufs=1, space="PSUM") as psC,  # pk|u
            tc.tile_pool(name="psD", bufs=4, space="PSUM") as psD,  # at chunks
        ):
            # ---- input DMAs: xqk on the scalar queue (free earliest),
            # wts on sync, vkt on gpsimd; act-table preload after the issue.
            nc.scalar.add_instruction(
                mybir.InstLoadActFuncSet(
                    name="preload_act_nle", ins=[], outs=[], act_func_set_id=6
                )
            )
            xqk = pin.tile([128, _S], bf16, tag="xqk")
            nc.scalar.dma_start(out=xqk, in_=xqk_d)
            wts = pin.tile([128, _WTSW], bf16, tag="wts")
            nc.sync.dma_start(out=wts, in_=wts_d)
            vkt = pin.tile([_D, 2 * _S], bf16, tag="vkt")
            nc.gpsimd.dma_start(out=vkt, in_=vkt_d)

            w1 = wts[:, _W1:_W1 + 128]
            w2 = wts[:, _W2:_W2 + 128]
            eqek = wts[:, _EQEK:_EQEK + 2]
            wh_t = wts[0:_D, _WH:_WH + _P]
            wv_t = wts[0:_D, _WV:_WV + _P]
            b1col = wts[:, _B1:_B1 + 1]
            nb2col = wts[:, _B1 + 1:_B1 + 2]
            w2s = wts[:, _W2S:_W2S + 2]
            ones_row = wts[0:1, _ONES:_ONES + 128]
            b2s_row = wts[0:1, _ONES + 128:_ONES + 130]

            # ---- PE: z1, phiq, u, pk (only need inputs) ----
            z1_p = psA.tile([128, _S], f32, tag="big")
            nc.tensor.matmul(out=z1_p, lhsT=w1, rhs=xqk, start=True, stop=True)

            phiq_p = psB.tile([_P, _S], f32, tag="mid")
            nc.tensor.matmul(out=phiq_p, lhsT=wh_t, rhs=xqk[0:_D, :],
                             start=True, stop=True)

            # ---- ICNN layer 1 on ACT: z1 = ln(1 + exp(x + b1)) ----
            e1 = pw.tile([128, _S], bf16, tag="e1")
            nc.scalar.activation(out=e1, in_=z1_p, func=AF.Exp, bias=b1col, scale=1.0)
            z1 = pw.tile([128, _S], bf16, tag="z1")
            nc.scalar.activation(out=z1, in_=e1, func=AF.Ln, bias=1.0, scale=1.0)

            # ---- layer 2: only l2 = ln(1 + e^-(x+b2)) is nonlinear; the
            # linear part of z2 flows into fq/gk via the w2sum matmuls.
            z2_p = psA.tile([128, _S], f32, tag="big")
            nc.tensor.matmul(out=z2_p, lhsT=w2, rhs=z1, start=True, stop=True)

            # pku: cols 0:128 = u chunks, 128:256 = pk chunks
            pku_p = psC.tile([128, 2 * 128], f32, tag="pku")
            for c in range(NCH):
                nc.tensor.matmul(
                    out=pku_p[:, c * _P:(c + 1) * _P],
                    lhsT=vkt[:, c * 128:(c + 1) * 128],
                    rhs=wv_t, start=True, stop=True,
                )
            for c in range(NCH):
                nc.tensor.matmul(
                    out=pku_p[:, 128 + c * _P:128 + (c + 1) * _P],
                    lhsT=vkt[:, _S + c * 128:_S + (c + 1) * 128],
                    rhs=wh_t, start=True, stop=True,
                )


            # phiq exp rides the ACT idle slot while the z2 matmul runs
            phiq = pw.tile([_P, _S], bf16, tag="phiq")
            nc.scalar.activation(out=phiq, in_=phiq_p, func=AF.Exp,
                                 bias=0.0, scale=1.0)

            e2n = pw.tile([128, _S], bf16, tag="e2n")
            nc.scalar.activation(out=e2n, in_=z2_p, func=AF.Exp, bias=nb2col,
                                 scale=-1.0)
            l2 = pw.tile([128, _S], bf16, tag="l2")
            nc.scalar.activation(out=l2, in_=e2n, func=AF.Ln, bias=1.0, scale=1.0)

            epku = pw.tile([128, 2 * 128], bf16, tag="epku")
            nc.scalar.activation(out=epku, in_=pku_p, func=AF.Exp,
                                 bias=0.0, scale=1.0)
            eu = epku[:, 0:128]
            pkp = epku[:, 128:256]

            # ---- fq/gk columns: fg[t, 2c+(0=fq,1=gk)] ----
            # per chunk: w2sum^T z1 + ones (x) b2sums + eqek^T l2
            # all z1-part matmuls are emitted before the l2-part ones so the
            # in-order PE queue doesn't stall on l2 mid-loop; the per-chunk
            # accumulation groups therefore interleave (safe on HW: PSUM
            # accumulate is per-instruction), hence skip_group_check.
            fg_p = psB.tile([128, 2 * NCH], f32, tag="mid")
            for c in range(NCH):
                nc.tensor.matmul(
                    out=fg_p[:, 2 * c:2 * c + 2],
                    lhsT=z1[:, c * 128:(c + 1) * 128],
                    rhs=w2s, start=True, stop=False,
                )
                nc.tensor.matmul(
                    out=fg_p[:, 2 * c:2 * c + 2],
                    lhsT=l2[:, c * 128:(c + 1) * 128],
                    rhs=eqek, start=False, stop=True,
                )

            # ---- m = max_t gk; gkm = gk - m; fqm = fq + m - ln(S) ----
            m1 = pw.tile([128, 1], f32, tag="m1")
            nc.vector.reduce_max(m1, fg_p[:, 1:2 * NCH:2], axis=AX)
            allmax = pw.tile([128, 1], f32, tag="allmax")
            nc.gpsimd.partition_all_reduce(allmax, m1, channels=128,
                                           reduce_op=ReduceOp.max)
            gkm = pw.tile([128, NCH], f32, tag="gkm")
            nc.vector.tensor_scalar(gkm, fg_p[:, 1:2 * NCH:2], allmax, None,
                                    op0=OP.subtract)
            fqm = pw.tile([128, NCH], f32, tag="fqm")
            nc.vector.tensor_scalar(fqm, fg_p[:, 0:2 * NCH:2], allmax, -_LN_S,
                                    op0=OP.add, op1=OP.add)

            # ---- pk' = pk_plain * exp(gk - m) ; M = sum_t pk' eu ----
            # sgm is one small ACT exp; the per-chunk scale runs on the DVE
            # (per-chunk tiles so writes don't WAR-serialize with PE reads)
            sgm = pw.tile([128, NCH], f32, tag="sgm")
            nc.scalar.activation(out=sgm, in_=gkm, func=AF.Exp,
                                 bias=0.0, scale=1.0)
            M_p = psB.tile([_P, _P], f32, tag="mid")
            pk2 = [pw.tile([128, _P], bf16, tag=f"pk2_{c}", name=f"pk2_{c}")
                   for c in range(NCH)]
            for c in range(NCH):
                nc.vector.tensor_scalar(
                    pk2[c], pkp[:, c * _P:(c + 1) * _P],
                    sgm[:, c:c + 1], None, op0=OP.mult,
                )
                nc.tensor.matmul(
                    out=M_p, lhsT=pk2[c], rhs=eu[:, c * _P:(c + 1) * _P],
                    start=(c == 0), stop=(c == NCH - 1),
                )
            M_sb = pw.tile([_P, _P], bf16, tag="M_sb")
            nc.vector.tensor_copy(out=M_sb, in_=M_p)

            # ---- A^T chunks, ln, + (fq + m - ln S), per-chunk tiles ----
            y_sb = pw.tile([128, NCH * _P], f32, tag="y")
            for c in range(NCH):
                at_c = psD.tile([128, _P], f32, tag="at", name=f"at_{c}")
                nc.tensor.matmul(
                    out=at_c,
                    lhsT=phiq[:, c * 128:(c + 1) * 128],
                    rhs=M_sb, start=True, stop=True,
                )
                lnA_c = pw.tile([128, _P], f32, tag=f"lnA_{c}", name=f"lnA_{c}")
                nc.scalar.activation(out=lnA_c, in_=at_c, func=AF.Ln,
                                     bias=0.0, scale=1.0)
                nc.vector.tensor_scalar(
                    y_sb[:, c * _P:(c + 1) * _P], lnA_c,
                    fqm[:, c:c + 1], None, op0=OP.add,
                )

            nc.sync.dma_start(out=y_d, in_=y_sb)

            if dump:
                for nm, t in [
                    ("d_z1", z1), ("d_l2", l2), ("d_eu", eu),
                    ("d_phiq", phiq), ("d_gkm", gkm), ("d_fqm", fqm),
                    ("d_Msb", M_sb), ("d_m1", m1), ("d_allmax", allmax),
                ]:
                    dd = nc.dram_tensor(nm, list(t.shape), t.dtype,
                                        kind="ExternalOutput").ap()
                    nc.sync.dma_start(out=dd, in_=t)

    if not nc.is_finalized():
        nc.finalize()
    return nc


def _host_inputs(q, k, v, spW1q, b1q, spW2q, b2q, spW1k, b1k, spW2k, b2k, Wh, Wv):
    """Build the per-core input maps (numpy layout prep only)."""
    import ml_dtypes

    bf16 = ml_dtypes.bfloat16
    S, D, P = _S, _D, _P

    wts = np.zeros((128, _WTSW), np.float32)
    wts[0:D, _W1:_W1 + D] = spW1q.T
    wts[D:2 * D, _W1 + D:_W1 + 2 * D] = spW1k.T
    wts[0:D, _W2:_W2 + D] = spW2q.T
    wts[D:2 * D, _W2 + D:_W2 + 2 * D] = spW2k.T
    wts[0:D, _EQEK] = 1.0
    wts[D:2 * D, _EQEK + 1] = 1.0
    wts[0:D, _WH:_WH + P] = Wh.T
    wts[0:D, _WV:_WV + P] = Wv.T
    wts[0:D, _B1] = b1q
    wts[D:2 * D, _B1] = b1k
    wts[0:D, _B1 + 1] = -b2q
    wts[D:2 * D, _B1 + 1] = -b2k
    wts[0:D, _W2S] = spW2q.sum(axis=0)      # column sums of W2q
    wts[D:2 * D, _W2S + 1] = spW2k.sum(axis=0)
    wts[0, _ONES:_ONES + 128] = 1.0
    wts[0, _ONES + 128] = float(np.sum(b2q))
    wts[0, _ONES + 129] = float(np.sum(b2k))
    wts = wts.astype(bf16)

    in_maps = []
    for h in range(_H):
        xqk = np.empty((128, S), np.float32)
        xqk[0:D] = q[0, h].T
        xqk[D:2 * D] = k[0, h].T
        vkt = np.empty((D, 2 * S), np.float32)
        vkt[:, 0:S] = v[0, h].T
        vkt[:, S:2 * S] = k[0, h].T  # kT copy at partitions 0:64 for pk lhsT
        in_maps.append(dict(
            wts=wts,
            xqk=xqk.astype(bf16),
            vkt=vkt.astype(bf16),
        ))
    return in_maps


def kernel(**inputs):
    from concourse.bass_utils import run_bass_kernel_spmd

    np_in = {k: np.asarray(v) for k, v in inputs.items()}
    q, k, v = np_in["q"], np_in["k"], np_in["v"]

    def sp(x):  # softplus for the small weight matrices (host prep)
        return np.log1p(np.exp(x.astype(np.float64))).astype(np.float32)

    in_maps = _host_inputs(
        q, k, v,
        sp(np_in["sq_raw1"]), np_in["sq_b1"], sp(np_in["sq_raw2"]), np_in["sq_b2"],
        sp(np_in["sk_raw1"]), np_in["sk_b1"], sp(np_in["sk_raw2"]), np_in["sk_b2"],
        np_in["Wh"], np_in["Wv"],
    )

    if "nc" not in _CACHE:
        _CACHE["nc"] = _build_bass()
    nc = _CACHE["nc"]

    res = run_bass_kernel_spmd(nc, in_maps, list(range(_NCORES)))
    out = np.zeros((_B, _H, _S, _P), np.float32)
    for h in range(_H):
        yt = res.results[h]["y"]  # [128, 4*P]; col block c = y[c*128:(c+1)*128, :]
        out[0, h] = yt.reshape(128, 4, _P).transpose(1, 0, 2).reshape(_S, _P)
    return out
